# revision 14
# baseline (speedup 1.0000x reference)
"""Trainium2 Bass kernel for STSBaselineNet (embed -> biLSTM -> max-pool).

Sharding: data-parallel over batch B=256 across 8 cores (32 sentences/core).
Per core:
  Phase A: indirect-DMA gather of embedding rows (bf16, padded/augmented to
           384 feats: col 300 = 1.0 bias lane, col 301 = pad flag), DMA-
           transpose to feature-major, input-projection matmuls into an
           SBUF-resident zx_all (token-major, both dirs).
  Phase B: 64-step recurrence, both dirs as independent pipelined chains.
           Layout: gates on partitions, batch on free dim.  Gate order
           [i, f, o, g] so one sigmoid covers cols 0:192.  Backward-dir pad
           masking is folded into the input projection via the pad-flag lane
           (-BIG on i/f/o logits => gates ~0 => state stays 0 through pad).
  Phase C: masked max over time (h_all + mbig, reduce), PE transpose, DMA out.
"""

import os
import numpy as np
import ml_dtypes

import concourse.bass as bass
import concourse.bacc as bacc
import concourse.mybir as mybir
import concourse.tile as tile
from concourse import bass_utils

V, E, HID, B, T = 50000, 300, 256, 256, 64
NCORES = 8
BC = B // NCORES            # 32 sentences/core
NTOK = BC * T               # 2048 tokens/core
NTT = NTOK // 128           # 16 gather tiles
EP = 384                    # padded feature dim (300 emb + bias + flag + 0pad)
BIGNEG = -30.0              # logit offset for gate masking
MAXNEG = -8.0               # mask offset for the final max (|h| < 1)

F32 = mybir.dt.float32
BF16 = mybir.dt.bfloat16
I32 = mybir.dt.int32
AF = mybir.ActivationFunctionType
OP = mybir.AluOpType

bf = ml_dtypes.bfloat16

# chunk order: gate blocks [i, f, o, g], each split into two 128-row halves.
# torch gate order in W is [i, f, g, o] (256 rows each).
GB_BASE = {0: 0, 1: 256, 2: 768, 3: 512}   # our block -> orig row base

_CACHE = {}
LAST_RESULTS = None


def _build_program():
    nc = bacc.Bacc(None, target_bir_lowering=False)

    emb_d = nc.dram_tensor("emb", [V, EP], BF16, kind="ExternalInput")
    idx_d = nc.dram_tensor("idx", [128, NTT], I32, kind="ExternalInput")
    mflag_d = nc.dram_tensor("mflag", [128, NTT], BF16, kind="ExternalInput")
    wstat_d = nc.dram_tensor("wstat", [128, 4096], BF16, kind="ExternalInput")
    wih_d = nc.dram_tensor("wih", [128, 6144], BF16, kind="ExternalInput")
    mbig_d = nc.dram_tensor("mbig", [128, 8192], BF16, kind="ExternalInput")
    out_d = nc.dram_tensor("out", [BC, 2 * HID], F32, kind="ExternalOutput")

    with tile.TileContext(nc) as tc:
        with (
            tc.tile_pool(name="const", bufs=1) as cpool,
            tc.tile_pool(name="work", bufs=4) as wpool,
            tc.tile_pool(name="psum", bufs=2, space="PSUM") as ppool,
            tc.tile_pool(name="psumz", bufs=2, space="PSUM") as zpool,
            tc.tile_pool(name="psumt", bufs=2, space="PSUM") as tpool,
        ):
            # ---- persistent SBUF tensors ----
            wstat_sb = cpool.tile([128, 4096], BF16, tag="wstat")
            wih_sb = cpool.tile([128, 6144], BF16, tag="wih")
            idx_sb = cpool.tile([128, NTT], I32, tag="idx")
            mflag_sb = cpool.tile([128, NTT], BF16, tag="mflag")
            mbig_sb = cpool.tile([128, 8192], BF16, tag="mbig")
            xg = cpool.tile([128, NTT * EP], BF16, tag="xg")        # gathered
            xt = cpool.tile([128, 3 * 2048], BF16, tag="xt")        # transposed
            zx = cpool.tile([128, T * 512], BF16, tag="zx")         # projections
            h_all = cpool.tile([128, 128 * T], BF16, tag="h_all")
            c_st = cpool.tile([128, 128], F32, tag="c_st")          # c (f|b)
            hzero = cpool.tile([128, 32], BF16, tag="hzero")
            ident = cpool.tile([128, 128], F32, tag="ident")
            ident_bf = cpool.tile([128, 128], BF16, tag="ident_bf")
            hmax = cpool.tile([128, 128], F32, tag="hmax")
            hmaxT = cpool.tile([128, 128], F32, tag="hmaxT")

            nc.sync.dma_start(out=wstat_sb[:], in_=wstat_d[:, :])
            nc.sync.dma_start(out=wih_sb[:], in_=wih_d[:, :])
            nc.sync.dma_start(out=idx_sb[:], in_=idx_d[:, :])
            nc.sync.dma_start(out=mflag_sb[:], in_=mflag_d[:, :])
            nc.sync.dma_start(out=mbig_sb[:], in_=mbig_d[:, :])

            nc.vector.memset(c_st[:], 0.0)
            nc.vector.memset(hzero[:], 0.0)
            from concourse.masks import make_identity
            make_identity(nc, ident[:])
            nc.vector.tensor_copy(out=ident_bf[:], in_=ident[:])

            # ---- Phase A: gather + augment + transpose ----
            for tk in range(NTT):
                nc.gpsimd.indirect_dma_start(
                    out=xg[:, tk * EP:(tk + 1) * EP],
                    out_offset=None,
                    in_=emb_d[:, :],
                    in_offset=bass.IndirectOffsetOnAxis(
                        ap=idx_sb[:, tk:tk + 1], axis=0),
                )
            for tk in range(NTT):
                nc.vector.memset(xg[:, tk * EP + 300:tk * EP + 301], 1.0)
                nc.vector.tensor_copy(
                    out=xg[:, tk * EP + 301:tk * EP + 302],
                    in_=mflag_sb[:, tk:tk + 1])
            for kb in range(3):
                for grp in range(4):
                    xtp = tpool.tile([128, 512], BF16, tag="xtp")
                    for q in range(4):
                        tk = grp * 4 + q
                        nc.tensor.transpose(
                            xtp[:, q * 128:(q + 1) * 128],
                            xg[:, tk * EP + kb * 128:tk * EP + (kb + 1) * 128],
                            ident_bf[:])
                    nc.vector.tensor_copy(
                        out=xt[:, kb * 2048 + grp * 512:kb * 2048 + (grp + 1) * 512],
                        in_=xtp[:])

            # ---- Phase A2: input projection into zx ----
            # zx col = t*512 + d*256 + ch*32 + b   (token-major)
            zx_v = zx[:].rearrange("p (t d c g) -> p t d c g", t=T, d=2, c=8)
            ncopy = 0
            for d in range(2):
                for ch in range(8):
                    for n in range(4):   # token groups of 512
                        zxp = ppool.tile([128, 512], F32, tag="zxp")
                        for kb in range(3):
                            nc.tensor.matmul(
                                zxp[:],
                                lhsT=wih_sb[:, ((d * 8 + ch) * 3 + kb) * 128:
                                            ((d * 8 + ch) * 3 + kb + 1) * 128],
                                rhs=xt[:, kb * 2048 + n * 512:kb * 2048 + (n + 1) * 512],
                                start=(kb == 0), stop=(kb == 2),
                            )
                        src = zxp[:].rearrange("p (tk b2 t) -> p tk b2 t",
                                               tk=4, b2=2)
                        dst = zx_v[:, :, d, ch, n * 8:(n + 1) * 8].rearrange(
                            "p t (tk b2) -> p tk b2 t", tk=4)
                        if ncopy % 2 == 0:
                            nc.vector.tensor_copy(out=dst, in_=src)
                        else:
                            nc.scalar.copy(out=dst, in_=src)
                        ncopy += 1

            # ---- Phase B: recurrence ----
            h_v = h_all[:].rearrange("p (j s) -> p j s", s=T)
            for s in range(T):
                for d in range(2):
                    t_in = s if d == 0 else T - 1 - s
                    zq = zpool.tile([128, 256], F32, tag=f"zq{d}")
                    for sl in range(8):
                        for k in range(2):
                            if s == 0:
                                rhs = hzero[:]
                            else:
                                rhs = h_v[:, d * 64 + k * 32:d * 64 + (k + 1) * 32,
                                          s - 1:s]
                            nc.tensor.matmul(
                                zq[:, sl * 32:(sl + 1) * 32],
                                lhsT=wstat_sb[:, ((d * 8 + sl) * 2 + k) * 128:
                                              ((d * 8 + sl) * 2 + k + 1) * 128],
                                rhs=rhs,
                                start=(k == 0), stop=(k == 1),
                            )
                    zs = wpool.tile([128, 256], F32, tag=f"zs{d}")
                    nc.vector.tensor_add(
                        zs[:], zq[:],
                        zx[:, t_in * 512 + d * 256:t_in * 512 + (d + 1) * 256])
                    sg = wpool.tile([128, 256], F32, tag=f"sg{d}")
                    nc.scalar.activation(sg[:, 0:192], zs[:, 0:192], AF.Sigmoid)
                    nc.scalar.activation(sg[:, 192:256], zs[:, 192:256], AF.Tanh)
                    tmp = wpool.tile([128, 64], F32, tag=f"tmp{d}")
                    nc.vector.tensor_mul(tmp[:], sg[:, 0:64], sg[:, 192:256])
                    nc.vector.tensor_mul(c_st[:, d * 64:(d + 1) * 64],
                                         sg[:, 64:128],
                                         c_st[:, d * 64:(d + 1) * 64])
                    nc.vector.tensor_add(c_st[:, d * 64:(d + 1) * 64],
                                         c_st[:, d * 64:(d + 1) * 64], tmp[:])
                    tch = wpool.tile([128, 64], F32, tag=f"tch{d}")
                    nc.scalar.activation(tch[:], c_st[:, d * 64:(d + 1) * 64],
                                         AF.Tanh)
                    hdst = h_v[:, d * 64:(d + 1) * 64, s:s + 1].rearrange(
                        "p j one -> p (j one)")
                    nc.vector.tensor_mul(hdst, sg[:, 128:192], tch[:])

            # ---- Phase C: masked max + output ----
            nc.vector.tensor_add(h_all[:], h_all[:], mbig_sb[:])
            nc.vector.tensor_reduce(hmax[:], h_v, axis=mybir.AxisListType.X,
                                    op=OP.max)
            tp = tpool.tile([128, 128], F32, tag="xtp")
            nc.tensor.transpose(tp[:], hmax[:], ident[:])
            nc.vector.tensor_copy(out=hmaxT[:], in_=tp[:])
            # dims (d, k, b, p) over out[b, d*256+k*128+p]
            out_ap = bass.AP(tensor=out_d[:, :].tensor, offset=0,
                             ap=[[256, 2], [128, 2], [512, BC], [1, 128]])
            nc.sync.dma_start(out=out_ap, in_=hmaxT[:])

    nc.finalize()
    return nc


def _host_prep(token_ids, lengths, emb, w_ih_f, w_hh_f, b_f, w_ih_b, w_hh_b,
               b_b):
    """Build per-core input maps (all numpy)."""
    emb384 = np.zeros((V, EP), dtype=bf)
    emb384[:, :E] = emb.astype(bf)

    # row selection for chunk ch=(gb, k_out): orig rows GB_BASE[gb]+k_out*128
    def sel_rows(ch):
        gb, ko = ch // 2, ch % 2
        base = GB_BASE[gb] + ko * 128
        return slice(base, base + 128)

    wstat = np.zeros((128, 4096), dtype=bf)
    whh = {0: w_hh_f, 1: w_hh_b}
    for d in range(2):
        for sl in range(8):
            for k in range(2):
                blk = whh[d][sel_rows(sl), k * 128:(k + 1) * 128].T  # [K,M]
                col = ((d * 8 + sl) * 2 + k) * 128
                wstat[:, col:col + 128] = blk.astype(bf)

    wih = np.zeros((128, 6144), dtype=bf)
    for d in range(2):
        w_ih = w_ih_f if d == 0 else w_ih_b
        bias = b_f if d == 0 else b_b
        aug = np.zeros((EP, 4 * HID), dtype=np.float32)
        aug[:E, :] = w_ih.T
        aug[300, :] = bias
        if d == 1:
            mv = np.zeros(4 * HID, dtype=np.float32)
            mv[0:512] = BIGNEG          # i, f
            mv[768:1024] = BIGNEG       # o
            aug[301, :] = mv
        for ch in range(8):
            for kb in range(3):
                blk = aug[kb * 128:(kb + 1) * 128, sel_rows(ch)]
                col = ((d * 8 + ch) * 3 + kb) * 128
                wih[:, col:col + 128] = blk.astype(bf)

    in_maps = []
    for c in range(NCORES):
        tok = token_ids[c * BC:(c + 1) * BC]      # [32, 64]
        ln = lengths[c * BC:(c + 1) * BC]         # [32]

        flat = tok.reshape(-1)                    # j = b*64 + t
        idx = flat.reshape(NTT, 128).T.astype(np.int32).copy()  # [128, NTT]

        tt = np.arange(T)[None, :]                # [1, 64]
        pad = (tt >= ln[:, None]).astype(np.float32)  # [32, 64] 1 if pad
        mflag = pad.reshape(-1).reshape(NTT, 128).T.astype(bf).copy()

        # mbig[p, j*64+s], j = d*64+k*32+b ; valid t: s (fwd) / 63-s (bwd)
        mb_ = np.zeros((128, 8192), dtype=np.float32)
        ss = np.arange(T)
        for d in range(2):
            tt_of_s = ss if d == 0 else T - 1 - ss
            invalid = (tt_of_s[None, :] >= ln[:, None])   # [32, 64]
            for k in range(2):
                for b_i in range(BC):
                    j = d * 64 + k * 32 + b_i
                    mb_[:, j * 64:(j + 1) * 64] = np.where(
                        invalid[b_i], MAXNEG, 0.0)[None, :]
        in_maps.append({
            "emb": emb384,
            "idx": idx,
            "mflag": mflag,
            "wstat": wstat,
            "wih": wih,
            "mbig": mb_.astype(bf),
        })
    return in_maps


def kernel(token_ids, lengths, emb, w_ih_f, w_hh_f, b_f, w_ih_b, w_hh_b, b_b):
    global LAST_RESULTS
    if "nc" not in _CACHE:
        _CACHE["nc"] = _build_program()
    nc = _CACHE["nc"]
    in_maps = _host_prep(token_ids, lengths, emb, w_ih_f, w_hh_f, b_f,
                         w_ih_b, w_hh_b, b_b)
    res = bass_utils.run_bass_kernel_spmd(nc, in_maps, list(range(NCORES)))
    LAST_RESULTS = res
    out = np.concatenate([res.results[c]["out"] for c in range(NCORES)],
                         axis=0)
    return out.astype(np.float32)


# revision 17
# speedup vs baseline: 3.0949x; 3.0949x over previous
"""Trainium2 Bass kernel for STSBaselineNet (embed -> biLSTM -> max-pool).

Sharding: data-parallel over batch B=256 across 8 cores (32 sentences/core).
Per core:
  Phase A: indirect-DMA gather of embedding rows (bf16, padded/augmented to
           384 feats: col 300 = 1.0 bias lane, col 301 = pad flag), DMA-
           transpose to feature-major, input-projection matmuls into an
           SBUF-resident zx_all (token-major, both dirs).
  Phase B: 64-step recurrence, both dirs as independent pipelined chains.
           Layout: gates on partitions, batch on free dim.  Gate order
           [i, f, o, g] so one sigmoid covers cols 0:192.  Backward-dir pad
           masking is folded into the input projection via the pad-flag lane
           (-BIG on i/f/o logits => gates ~0 => state stays 0 through pad).
  Phase C: masked max over time (h_all + mbig, reduce), PE transpose, DMA out.
"""

import os
import numpy as np
import ml_dtypes

import concourse.bass as bass
import concourse.bacc as bacc
import concourse.mybir as mybir
import concourse.tile as tile
from concourse import bass_utils

V, E, HID, B, T = 50000, 300, 256, 256, 64
NCORES = 8
BC = B // NCORES            # 32 sentences/core
NTOK = BC * T               # 2048 tokens/core
NTT = NTOK // 128           # 16 gather tiles
EP = 384                    # padded feature dim (300 emb + bias + flag + 0pad)
BIGNEG = -30.0              # logit offset for gate masking
MAXNEG = -8.0               # mask offset for the final max (|h| < 1)

F32 = mybir.dt.float32
BF16 = mybir.dt.bfloat16
I32 = mybir.dt.int32
AF = mybir.ActivationFunctionType
OP = mybir.AluOpType

bf = ml_dtypes.bfloat16

# chunk order: gate blocks [i, f, o, g], each split into two 128-row halves.
# torch gate order in W is [i, f, g, o] (256 rows each).
GB_BASE = {0: 0, 1: 256, 2: 768, 3: 512}   # our block -> orig row base

_CACHE = {}
LAST_RESULTS = None


def _build_program(stub_recur=False, stub_prolog=False):
    nc = bacc.Bacc(None, target_bir_lowering=False)

    emb_d = nc.dram_tensor("emb", [V, EP], BF16, kind="ExternalInput")
    idx_d = nc.dram_tensor("idx", [128, NTT], I32, kind="ExternalInput")
    mflag_d = nc.dram_tensor("mflag", [128, NTT], BF16, kind="ExternalInput")
    wstat_d = nc.dram_tensor("wstat", [128, 4096], BF16, kind="ExternalInput")
    wih_d = nc.dram_tensor("wih", [128, 6144], BF16, kind="ExternalInput")
    mbig_d = nc.dram_tensor("mbig", [128, 8192], BF16, kind="ExternalInput")
    out_d = nc.dram_tensor("out", [BC, 2 * HID], F32, kind="ExternalOutput")

    with tile.TileContext(nc) as tc:
        with (
            tc.tile_pool(name="const", bufs=1) as cpool,
            tc.tile_pool(name="work", bufs=4) as wpool,
            tc.tile_pool(name="psum", bufs=2, space="PSUM") as ppool,
            tc.tile_pool(name="psumz", bufs=2, space="PSUM") as zpool,
            tc.tile_pool(name="psumt", bufs=2, space="PSUM") as tpool,
        ):
            # ---- persistent SBUF tensors ----
            wstat_sb = cpool.tile([128, 4096], BF16, tag="wstat")
            wih_sb = cpool.tile([128, 6144], BF16, tag="wih")
            idx_sb = cpool.tile([128, NTT], I32, tag="idx")
            mflag_sb = cpool.tile([128, NTT], BF16, tag="mflag")
            mbig_sb = cpool.tile([128, 8192], BF16, tag="mbig")
            xg = cpool.tile([128, NTT * EP], BF16, tag="xg")        # gathered
            xt = cpool.tile([128, 3 * 2048], BF16, tag="xt")        # transposed
            zx = cpool.tile([128, T * 512], BF16, tag="zx")         # projections
            h_all = cpool.tile([128, 128 * T], BF16, tag="h_all")
            c_st = cpool.tile([128, 128], F32, tag="c_st")          # c (f|b)
            hzero = cpool.tile([128, 32], BF16, tag="hzero")
            ident = cpool.tile([128, 128], F32, tag="ident")
            ident_bf = cpool.tile([128, 128], BF16, tag="ident_bf")
            hmax = cpool.tile([128, 128], F32, tag="hmax")
            hmaxT = cpool.tile([128, 128], F32, tag="hmaxT")

            nc.sync.dma_start(out=wstat_sb[:], in_=wstat_d[:, :])
            nc.sync.dma_start(out=wih_sb[:], in_=wih_d[:, :])
            nc.sync.dma_start(out=idx_sb[:], in_=idx_d[:, :])
            nc.sync.dma_start(out=mflag_sb[:], in_=mflag_d[:, :])
            nc.sync.dma_start(out=mbig_sb[:], in_=mbig_d[:, :])

            nc.vector.memset(c_st[:], 0.0)
            nc.vector.memset(hzero[:], 0.0)
            from concourse.masks import make_identity
            make_identity(nc, ident[:])
            nc.vector.tensor_copy(out=ident_bf[:], in_=ident[:])

            # ---- Phase A: gather + augment + transpose ----
            for tk in range(NTT if not stub_prolog else 0):
                nc.gpsimd.indirect_dma_start(
                    out=xg[:, tk * EP:(tk + 1) * EP],
                    out_offset=None,
                    in_=emb_d[:, :],
                    in_offset=bass.IndirectOffsetOnAxis(
                        ap=idx_sb[:, tk:tk + 1], axis=0),
                )
            for tk in range(NTT if not stub_prolog else 0):
                nc.vector.memset(xg[:, tk * EP + 300:tk * EP + 301], 1.0)
                nc.vector.tensor_copy(
                    out=xg[:, tk * EP + 301:tk * EP + 302],
                    in_=mflag_sb[:, tk:tk + 1])
            for kb in range(3 if not stub_prolog else 0):
                for grp in range(4):
                    xtp = tpool.tile([128, 512], BF16, tag="xtp")
                    for q in range(4):
                        tk = grp * 4 + q
                        nc.tensor.transpose(
                            xtp[:, q * 128:(q + 1) * 128],
                            xg[:, tk * EP + kb * 128:tk * EP + (kb + 1) * 128],
                            ident_bf[:])
                    nc.vector.tensor_copy(
                        out=xt[:, kb * 2048 + grp * 512:kb * 2048 + (grp + 1) * 512],
                        in_=xtp[:])

            # ---- Phase A2: input projection into zx ----
            # zx col = t*512 + d*256 + ch*32 + b   (token-major)
            zx_v = zx[:].rearrange("p (t d c g) -> p t d c g", t=T, d=2, c=8)
            ncopy = 0
            for d in range(2 if not stub_prolog else 0):
                for ch in range(8):
                    for n in range(4):   # token groups of 512
                        zxp = ppool.tile([128, 512], F32, tag="zxp")
                        for kb in range(3):
                            nc.tensor.matmul(
                                zxp[:],
                                lhsT=wih_sb[:, ((d * 8 + ch) * 3 + kb) * 128:
                                            ((d * 8 + ch) * 3 + kb + 1) * 128],
                                rhs=xt[:, kb * 2048 + n * 512:kb * 2048 + (n + 1) * 512],
                                start=(kb == 0), stop=(kb == 2),
                            )
                        src = zxp[:].rearrange("p (tk b2 t) -> p tk b2 t",
                                               tk=4, b2=2)
                        dst = zx_v[:, :, d, ch, n * 8:(n + 1) * 8].rearrange(
                            "p t (tk b2) -> p tk b2 t", tk=4)
                        if ncopy % 2 == 0:
                            nc.vector.tensor_copy(out=dst, in_=src)
                        else:
                            nc.scalar.copy(out=dst, in_=src)
                        ncopy += 1

            # ---- Phase B: recurrence ----
            h_v = h_all[:].rearrange("p (j s) -> p j s", s=T)
            for s in range(T if not stub_recur else 0):
                for d in range(2):
                    t_in = s if d == 0 else T - 1 - s
                    zq = zpool.tile([128, 256], F32, tag=f"zq{d}")
                    for sl in range(8):
                        for k in range(2):
                            if s == 0:
                                rhs = hzero[:]
                            else:
                                rhs = h_v[:, d * 64 + k * 32:d * 64 + (k + 1) * 32,
                                          s - 1:s]
                            nc.tensor.matmul(
                                zq[:, sl * 32:(sl + 1) * 32],
                                lhsT=wstat_sb[:, ((d * 8 + sl) * 2 + k) * 128:
                                              ((d * 8 + sl) * 2 + k + 1) * 128],
                                rhs=rhs,
                                start=(k == 0), stop=(k == 1),
                            )
                    zs = wpool.tile([128, 256], F32, tag=f"zs{d}")
                    nc.vector.tensor_add(
                        zs[:], zq[:],
                        zx[:, t_in * 512 + d * 256:t_in * 512 + (d + 1) * 256])
                    sg = wpool.tile([128, 256], F32, tag=f"sg{d}")
                    nc.scalar.activation(sg[:, 0:192], zs[:, 0:192], AF.Sigmoid)
                    nc.scalar.activation(sg[:, 192:256], zs[:, 192:256], AF.Tanh)
                    tmp = wpool.tile([128, 64], F32, tag=f"tmp{d}")
                    nc.vector.tensor_mul(tmp[:], sg[:, 0:64], sg[:, 192:256])
                    nc.vector.tensor_mul(c_st[:, d * 64:(d + 1) * 64],
                                         sg[:, 64:128],
                                         c_st[:, d * 64:(d + 1) * 64])
                    nc.vector.tensor_add(c_st[:, d * 64:(d + 1) * 64],
                                         c_st[:, d * 64:(d + 1) * 64], tmp[:])
                    tch = wpool.tile([128, 64], F32, tag=f"tch{d}")
                    nc.scalar.activation(tch[:], c_st[:, d * 64:(d + 1) * 64],
                                         AF.Tanh)
                    hdst = h_v[:, d * 64:(d + 1) * 64, s:s + 1].rearrange(
                        "p j one -> p (j one)")
                    nc.vector.tensor_mul(hdst, sg[:, 128:192], tch[:])

            # ---- Phase C: masked max + output ----
            nc.vector.tensor_add(h_all[:], h_all[:], mbig_sb[:])
            nc.vector.tensor_reduce(hmax[:], h_v, axis=mybir.AxisListType.X,
                                    op=OP.max)
            tp = tpool.tile([128, 128], F32, tag="xtp")
            nc.tensor.transpose(tp[:], hmax[:], ident[:])
            nc.vector.tensor_copy(out=hmaxT[:], in_=tp[:])
            # dims (d, k, b, p) over out[b, d*256+k*128+p]
            out_ap = bass.AP(tensor=out_d[:, :].tensor, offset=0,
                             ap=[[256, 2], [128, 2], [512, BC], [1, 128]])
            nc.sync.dma_start(out=out_ap, in_=hmaxT[:])

    nc.finalize()
    return nc


def _host_prep(token_ids, lengths, emb, w_ih_f, w_hh_f, b_f, w_ih_b, w_hh_b,
               b_b):
    """Build per-core input maps (all numpy)."""
    emb384 = np.zeros((V, EP), dtype=bf)
    emb384[:, :E] = emb.astype(bf)

    # row selection for chunk ch=(gb, k_out): orig rows GB_BASE[gb]+k_out*128
    def sel_rows(ch):
        gb, ko = ch // 2, ch % 2
        base = GB_BASE[gb] + ko * 128
        return slice(base, base + 128)

    wstat = np.zeros((128, 4096), dtype=bf)
    whh = {0: w_hh_f, 1: w_hh_b}
    for d in range(2):
        for sl in range(8):
            for k in range(2):
                blk = whh[d][sel_rows(sl), k * 128:(k + 1) * 128].T  # [K,M]
                col = ((d * 8 + sl) * 2 + k) * 128
                wstat[:, col:col + 128] = blk.astype(bf)

    wih = np.zeros((128, 6144), dtype=bf)
    for d in range(2):
        w_ih = w_ih_f if d == 0 else w_ih_b
        bias = b_f if d == 0 else b_b
        aug = np.zeros((EP, 4 * HID), dtype=np.float32)
        aug[:E, :] = w_ih.T
        aug[300, :] = bias
        if d == 1:
            mv = np.zeros(4 * HID, dtype=np.float32)
            mv[0:512] = BIGNEG          # i, f
            mv[768:1024] = BIGNEG       # o
            aug[301, :] = mv
        for ch in range(8):
            for kb in range(3):
                blk = aug[kb * 128:(kb + 1) * 128, sel_rows(ch)]
                col = ((d * 8 + ch) * 3 + kb) * 128
                wih[:, col:col + 128] = blk.astype(bf)

    in_maps = []
    for c in range(NCORES):
        tok = token_ids[c * BC:(c + 1) * BC]      # [32, 64]
        ln = lengths[c * BC:(c + 1) * BC]         # [32]

        flat = tok.reshape(-1)                    # j = b*64 + t
        idx = flat.reshape(NTT, 128).T.astype(np.int32).copy()  # [128, NTT]

        tt = np.arange(T)[None, :]                # [1, 64]
        pad = (tt >= ln[:, None]).astype(np.float32)  # [32, 64] 1 if pad
        mflag = pad.reshape(-1).reshape(NTT, 128).T.astype(bf).copy()

        # mbig[p, j*64+s], j = d*64+k*32+b ; valid t: s (fwd) / 63-s (bwd)
        mb_ = np.zeros((128, 8192), dtype=np.float32)
        ss = np.arange(T)
        for d in range(2):
            tt_of_s = ss if d == 0 else T - 1 - ss
            invalid = (tt_of_s[None, :] >= ln[:, None])   # [32, 64]
            for k in range(2):
                for b_i in range(BC):
                    j = d * 64 + k * 32 + b_i
                    mb_[:, j * 64:(j + 1) * 64] = np.where(
                        invalid[b_i], MAXNEG, 0.0)[None, :]
        in_maps.append({
            "emb": emb384,
            "idx": idx,
            "mflag": mflag,
            "wstat": wstat,
            "wih": wih,
            "mbig": mb_.astype(bf),
        })
    return in_maps


def kernel(token_ids, lengths, emb, w_ih_f, w_hh_f, b_f, w_ih_b, w_hh_b, b_b):
    global LAST_RESULTS
    if "nc" not in _CACHE:
        _CACHE["nc"] = _build_program()
    nc = _CACHE["nc"]
    in_maps = _host_prep(token_ids, lengths, emb, w_ih_f, w_hh_f, b_f,
                         w_ih_b, w_hh_b, b_b)
    res = bass_utils.run_bass_kernel_spmd(nc, in_maps, list(range(NCORES)))
    LAST_RESULTS = res
    out = np.concatenate([res.results[c]["out"] for c in range(NCORES)],
                         axis=0)
    return out.astype(np.float32)
